# revision 4
# baseline (speedup 1.0000x reference)
"""GATv2 layer (broadcast-score variant) as a Bass/Tile kernel on 8 NeuronCores.

Math: since scores[i,j] = e[j] (row-broadcast) masked by A, the masked softmax +
aggregation collapse to
    g = exp(e),  e = relu(X @ W.T) @ a_w
    out = relu( (A @ (g*Wh)) / (A @ g) )          with Wh = X @ W.T
Each core computes a 1024-row block of the output:
  phase 1 (replicated): Wh, e, g, G = [g*Wh | g]  ([8192, 129])
  phase 2 (sharded):    acc = A_block @ G  via PE, contraction j on partitions,
                        using the host-transposed A.T block as lhsT.
"""

import numpy as np

import concourse.tile as tile
from concourse import bacc, mybir
from concourse.bass_utils import run_bass_kernel_spmd

N, IN_DIM, OUT_DIM = 8192, 256, 128
NCORES = 8
RPC = N // NCORES          # rows per core (1024)
P = 128                    # partitions
NJ = N // P                # 64 contraction chunks
NI = RPC // P              # 8 output row-tiles per core
DH = IN_DIM // P           # 2 chunks of the d-contraction
F32 = mybir.dt.float32
AFT = mybir.ActivationFunctionType
GW = OUT_DIM + 1           # 129 G columns (gWh | g)
GPAD = 132                 # padded G row pitch


def build_nc():
    nc = bacc.Bacc("TRN2", target_bir_lowering=False)
    at = nc.dram_tensor("at", [N, RPC], F32, kind="ExternalInput")        # A.T col-block
    xt = nc.dram_tensor("xt", [IN_DIM, N], F32, kind="ExternalInput")     # X.T (full)
    wt = nc.dram_tensor("wt", [IN_DIM, OUT_DIM], F32, kind="ExternalInput")  # W.T
    awr = nc.dram_tensor("awr", [P, OUT_DIM], F32, kind="ExternalInput")  # a_w replicated
    out = nc.dram_tensor("out", [RPC, OUT_DIM], F32, kind="ExternalOutput")

    with tile.TileContext(nc) as tc:
        with (
            tc.tile_pool(name="big", bufs=1) as big,
            tc.tile_pool(name="atp", bufs=4) as atp,
            tc.tile_pool(name="ph1", bufs=4) as ph1,
            tc.tile_pool(name="outp", bufs=2) as outp,
        ):
            xt_sb = big.tile([P, DH, N], F32)
            nc.sync.dma_start(out=xt_sb, in_=xt.rearrange("(dh p) n -> p dh n", p=P))
            wt_sb = big.tile([P, DH, OUT_DIM], F32)
            nc.sync.dma_start(out=wt_sb, in_=wt.rearrange("(dh p) o -> p dh o", p=P))
            aw_sb = big.tile([P, OUT_DIM], F32)
            nc.sync.dma_start(out=aw_sb, in_=awr[:, :])
            G = big.tile([P, NJ, GPAD], F32)

            # ---- phase 1: Wh tiles -> e -> g -> G = [g*Wh | g] ----
            with tc.tile_pool(name="ps1", bufs=2, space="PSUM") as ps1:
                for t in range(NJ):
                    wh_ps = ps1.tile([P, OUT_DIM], F32)
                    for dh in range(DH):
                        nc.tensor.matmul(
                            wh_ps,
                            xt_sb[:, dh, t * P:(t + 1) * P],
                            wt_sb[:, dh, :],
                            start=(dh == 0),
                            stop=(dh == DH - 1),
                        )
                    r_sb = ph1.tile([P, OUT_DIM], F32)
                    nc.scalar.activation(r_sb, wh_ps, AFT.Relu)
                    m_sb = ph1.tile([P, OUT_DIM], F32)
                    nc.vector.tensor_mul(m_sb, r_sb, aw_sb)
                    e_col = ph1.tile([P, 1], F32)
                    nc.vector.reduce_sum(e_col, m_sb, axis=mybir.AxisListType.X)
                    nc.scalar.activation(G[:, t, OUT_DIM:GW], e_col, AFT.Exp)
                    nc.vector.tensor_scalar_mul(
                        G[:, t, 0:OUT_DIM], wh_ps, G[:, t, OUT_DIM:GW]
                    )

            # ---- phase 2: acc[i] += AT[jc, i-tile].T @ G[jc]  over all jc ----
            with tc.tile_pool(name="ps2", bufs=1, space="PSUM") as ps2:
                accs = [
                    ps2.tile([P, GW], F32, tag=f"acc{i}", name=f"acc{i}")
                    for i in range(NI)
                ]
                for c in range(NJ):
                    at_sb = atp.tile([P, RPC], F32)
                    nc.sync.dma_start(out=at_sb, in_=at[c * P:(c + 1) * P, :])
                    for i in range(NI):
                        nc.tensor.matmul(
                            accs[i],
                            at_sb[:, i * P:(i + 1) * P],
                            G[:, c, 0:GW],
                            start=(c == 0),
                            stop=(c == NJ - 1),
                        )
                for i in range(NI):
                    rec = outp.tile([P, 1], F32, tag="rec", name="rec")
                    nc.vector.reciprocal(rec, accs[i][:, OUT_DIM:GW])
                    o_sb = outp.tile([P, OUT_DIM], F32, tag="osb", name="osb")
                    nc.scalar.activation(
                        o_sb, accs[i][:, 0:OUT_DIM], AFT.Relu, scale=rec
                    )
                    nc.sync.dma_start(out=out[i * P:(i + 1) * P, :], in_=o_sb)
    nc.compile()
    return nc


_NC_CACHE = None


def _get_nc():
    global _NC_CACHE
    if _NC_CACHE is None:
        _NC_CACHE = build_nc()
    return _NC_CACHE


def kernel_with_results(X, A, W, a_w, trace=False):
    X = np.ascontiguousarray(np.asarray(X, dtype=np.float32))
    A = np.ascontiguousarray(np.asarray(A, dtype=np.float32))
    W = np.ascontiguousarray(np.asarray(W, dtype=np.float32))
    a_w = np.ascontiguousarray(np.asarray(a_w, dtype=np.float32))

    xt = np.ascontiguousarray(X.T)                       # [256, 8192]
    wt = np.ascontiguousarray(W.T)                       # [256, 128]
    awr = np.ascontiguousarray(np.broadcast_to(a_w[None, :], (P, OUT_DIM)))

    in_maps = []
    for c in range(NCORES):
        atb = np.ascontiguousarray(A[c * RPC:(c + 1) * RPC, :].T)  # [8192, 1024]
        in_maps.append({"at": atb, "xt": xt, "wt": wt, "awr": awr})

    res = run_bass_kernel_spmd(_get_nc(), in_maps, list(range(NCORES)), trace=trace)
    out = np.concatenate([r["out"] for r in res.results], axis=0)
    return out.astype(np.float32), res


def kernel(X, A, W, a_w):
    out, _ = kernel_with_results(X, A, W, a_w)
    return out


# revision 5
# speedup vs baseline: 2.4674x; 2.4674x over previous
"""GATv2 layer (broadcast-score variant) as a Bass/Tile kernel on 8 NeuronCores.

Math: since scores[i,j] = e[j] (row-broadcast) masked by A, the masked softmax +
aggregation collapse to
    g = exp(e),  e = relu(X @ W.T) @ a_w
    out = relu( (A @ (g*Wh)) / (A @ g) )          with Wh = X @ W.T
Each core computes a 1024-row block of the output:
  phase 1 (replicated): Wh, e, g, G = [g*Wh | g]  ([8192, 129])
  phase 2 (sharded):    acc = A_block @ G  via PE, contraction j on partitions,
                        using the host-transposed A.T block as lhsT.
"""

import numpy as np

import concourse.tile as tile
from concourse import bacc, mybir
from concourse.bass_utils import run_bass_kernel_spmd

N, IN_DIM, OUT_DIM = 8192, 256, 128
NCORES = 8
RPC = N // NCORES          # rows per core (1024)
P = 128                    # partitions
NJ = N // P                # 64 contraction chunks
NI = RPC // P              # 8 output row-tiles per core
DH = IN_DIM // P           # 2 chunks of the d-contraction
F32 = mybir.dt.float32
AFT = mybir.ActivationFunctionType
GW = OUT_DIM + 1           # 129 G columns (gWh | g)
GPAD = 132                 # padded G row pitch


def emit_body(nc, tc, io, pools):
    at, xt, wt, awr, out = io
    big, atp, ph1, outp = pools

    xt_sb = big.tile([P, DH, N], F32, tag="xt_sb", name="xt_sb")
    nc.sync.dma_start(out=xt_sb, in_=xt.rearrange("(dh p) n -> p dh n", p=P))
    wt_sb = big.tile([P, DH, OUT_DIM], F32, tag="wt_sb", name="wt_sb")
    nc.sync.dma_start(out=wt_sb, in_=wt.rearrange("(dh p) o -> p dh o", p=P))
    aw_sb = big.tile([P, OUT_DIM], F32, tag="aw_sb", name="aw_sb")
    nc.sync.dma_start(out=aw_sb, in_=awr[:, :])
    G = big.tile([P, NJ, GPAD], F32, tag="G", name="G")

    # ---- phase 1: Wh tiles -> e -> g -> G = [g*Wh | g] ----
    with tc.tile_pool(name="ps1", bufs=2, space="PSUM") as ps1:
        for t in range(NJ):
            wh_ps = ps1.tile([P, OUT_DIM], F32, name="wh_ps")
            for dh in range(DH):
                nc.tensor.matmul(
                    wh_ps,
                    xt_sb[:, dh, t * P:(t + 1) * P],
                    wt_sb[:, dh, :],
                    start=(dh == 0),
                    stop=(dh == DH - 1),
                )
            r_sb = ph1.tile([P, OUT_DIM], F32, name="r_sb")
            nc.scalar.activation(r_sb, wh_ps, AFT.Relu)
            m_sb = ph1.tile([P, OUT_DIM], F32, name="m_sb")
            nc.vector.tensor_mul(m_sb, r_sb, aw_sb)
            e_col = ph1.tile([P, 1], F32, name="e_col")
            nc.vector.reduce_sum(e_col, m_sb, axis=mybir.AxisListType.X)
            nc.scalar.activation(G[:, t, OUT_DIM:GW], e_col, AFT.Exp)
            nc.vector.tensor_scalar_mul(
                G[:, t, 0:OUT_DIM], wh_ps, G[:, t, OUT_DIM:GW]
            )

    # ---- phase 2: acc[i] += AT[jc, i-tile].T @ G[jc]  over all jc ----
    with tc.tile_pool(name="ps2", bufs=1, space="PSUM") as ps2:
        accs = [
            ps2.tile([P, GW], F32, tag=f"acc{i}", name=f"acc{i}")
            for i in range(NI)
        ]
        for c in range(NJ):
            at_sb = atp.tile([P, RPC], F32, tag="at_sb", name="at_sb")
            nc.sync.dma_start(out=at_sb, in_=at[c * P:(c + 1) * P, :])
            for i in range(NI):
                nc.tensor.matmul(
                    accs[i],
                    at_sb[:, i * P:(i + 1) * P],
                    G[:, c, 0:GW],
                    start=(c == 0),
                    stop=(c == NJ - 1),
                )
        for i in range(NI):
            rec = outp.tile([P, 1], F32, tag="rec", name="rec")
            nc.vector.reciprocal(rec, accs[i][:, OUT_DIM:GW])
            o_sb = outp.tile([P, OUT_DIM], F32, tag="osb", name="osb")
            nc.scalar.activation(
                o_sb, accs[i][:, 0:OUT_DIM], AFT.Relu, scale=rec
            )
            nc.sync.dma_start(out=out[i * P:(i + 1) * P, :], in_=o_sb)


def build_nc(repeat=1):
    nc = bacc.Bacc("TRN2", target_bir_lowering=False)
    at = nc.dram_tensor("at", [N, RPC], F32, kind="ExternalInput")        # A.T col-block
    xt = nc.dram_tensor("xt", [IN_DIM, N], F32, kind="ExternalInput")     # X.T (full)
    wt = nc.dram_tensor("wt", [IN_DIM, OUT_DIM], F32, kind="ExternalInput")  # W.T
    awr = nc.dram_tensor("awr", [P, OUT_DIM], F32, kind="ExternalInput")  # a_w replicated
    out = nc.dram_tensor("out", [RPC, OUT_DIM], F32, kind="ExternalOutput")

    with tile.TileContext(nc) as tc:
        with (
            tc.tile_pool(name="big", bufs=1) as big,
            tc.tile_pool(name="atp", bufs=4) as atp,
            tc.tile_pool(name="ph1", bufs=4) as ph1,
            tc.tile_pool(name="outp", bufs=2) as outp,
        ):
            for _ in range(repeat):
                emit_body(nc, tc, (at, xt, wt, awr, out), (big, atp, ph1, outp))
    nc.compile()
    return nc


_NC_CACHE = None


def _get_nc():
    global _NC_CACHE
    if _NC_CACHE is None:
        _NC_CACHE = build_nc()
    return _NC_CACHE


def make_in_maps(X, A, W, a_w):
    X = np.ascontiguousarray(np.asarray(X, dtype=np.float32))
    A = np.ascontiguousarray(np.asarray(A, dtype=np.float32))
    W = np.ascontiguousarray(np.asarray(W, dtype=np.float32))
    a_w = np.ascontiguousarray(np.asarray(a_w, dtype=np.float32))

    xt = np.ascontiguousarray(X.T)                       # [256, 8192]
    wt = np.ascontiguousarray(W.T)                       # [256, 128]
    awr = np.ascontiguousarray(np.broadcast_to(a_w[None, :], (P, OUT_DIM)))

    in_maps = []
    for c in range(NCORES):
        atb = np.ascontiguousarray(A[c * RPC:(c + 1) * RPC, :].T)  # [8192, 1024]
        in_maps.append({"at": atb, "xt": xt, "wt": wt, "awr": awr})
    return in_maps


def kernel_with_results(X, A, W, a_w, trace=False):
    in_maps = make_in_maps(X, A, W, a_w)
    res = run_bass_kernel_spmd(_get_nc(), in_maps, list(range(NCORES)), trace=trace)
    out = np.concatenate([r["out"] for r in res.results], axis=0)
    return out.astype(np.float32), res


def kernel(X, A, W, a_w):
    out, _ = kernel_with_results(X, A, W, a_w)
    return out


# revision 29
# speedup vs baseline: 9.0106x; 3.6519x over previous
"""GATv2 layer (broadcast-score variant) as a Bass/Tile kernel on 8 NeuronCores.

Math: since scores[i,j] = e[j] (row-broadcast) masked by A, the masked softmax +
aggregation collapse to
    g = exp(e),  e = relu(X @ W.T) @ a_w
    out = relu( (A @ (g*Wh)) / (A @ g) )          with Wh = X @ W.T
Each core computes a 1024-row block of the output:
  phase 1 (replicated): Wh, e, g, G = [g*Wh | g]  ([8192, 129])
  phase 2 (sharded):    acc = A_block @ G  via PE, contraction j on partitions,
                        using the host-transposed A.T block as lhsT.
"""

import numpy as np

import concourse.tile as tile
from concourse import bacc, mybir
from concourse.bass_utils import run_bass_kernel_spmd

N, IN_DIM, OUT_DIM = 8192, 256, 128
NCORES = 8
RPC = N // NCORES          # rows per core (1024)
P = 128                    # partitions
NJ = N // P                # 64 contraction chunks
NI = RPC // P              # 8 output row-tiles per core
DH = IN_DIM // P           # 2 chunks of the d-contraction
import os

F32 = mybir.dt.float32
F32R = mybir.dt.float32r   # TF32-like: 1 cyc/row on PE when moving dim >= 256
BF16 = mybir.dt.bfloat16
# phase-2 matmul dtype: f32r (default), f32, bf16
PH2_DT = {"f32r": F32R, "f32": F32, "bf16": BF16}[os.environ.get("PH2_DT", "f32r")]
AFT = mybir.ActivationFunctionType
GW = OUT_DIM + 1           # 129 G columns (gWh | g)
GP = 132                   # G pitch (16B aligned)
B1 = 4                     # phase-1 j-tile batch
NB = NJ // B1
HF = RPC // 2              # 512-wide i-halves for phase-2 streams


def emit_body(nc, tc, io, pools):
    at, xt, wt, awr, out = io
    big, atp, ph1, outp = pools

    xt_sb = big.tile([P, DH, N], F32, tag="xt_sb", name="xt_sb")
    nc.sync.dma_start(out=xt_sb, in_=xt.rearrange("(dh p) n -> p dh n", p=P))
    wt_sb = big.tile([P, DH, OUT_DIM], F32, tag="wt_sb", name="wt_sb")
    nc.sync.dma_start(out=wt_sb, in_=wt.rearrange("(dh p) o -> p dh o", p=P))
    aw_sb = big.tile([P, B1, OUT_DIM], F32, tag="aw_sb", name="aw_sb")
    nc.sync.dma_start(out=aw_sb, in_=awr.rearrange("p (b o) -> p b o", b=B1))
    G = big.tile([P, NJ, GP], PH2_DT, tag="G", name="G")
    ones = big.tile([1, P], F32, tag="ones", name="ones")
    nc.vector.memset(ones, 1.0)

    with tc.tile_pool(name="ps", bufs=1, space="PSUM") as ps:
        # ---- phase 1: Wh tiles -> e -> g -> G = [g*Wh | g], batched by B1 ----
        for b in range(NB):
            wh4 = ps.tile([P, B1, OUT_DIM], F32, tag="wh4", name="wh4", bufs=2)
            for k in range(B1):
                t = b * B1 + k
                for dh in range(DH):
                    nc.tensor.matmul(
                        wh4[:, k, :],
                        xt_sb[:, dh, t * P:(t + 1) * P],
                        wt_sb[:, dh, :],
                        start=(dh == 0),
                        stop=(dh == DH - 1),
                    )
            r4 = ph1.tile([P, B1, OUT_DIM], F32, name="r4")
            nc.scalar.activation(r4, wh4, AFT.Relu)
            m4 = ph1.tile([P, B1, OUT_DIM], F32, name="m4")
            nc.vector.tensor_mul(m4, r4, aw_sb)
            e4 = ph1.tile([P, B1], F32, name="e4")
            nc.vector.reduce_sum(e4, m4, axis=mybir.AxisListType.X)
            g4 = ph1.tile([P, B1], F32, name="g4")
            nc.scalar.activation(g4, e4, AFT.Exp)
            for k in range(B1):
                t = b * B1 + k
                nc.vector.tensor_scalar_mul(
                    G[:, t, 0:OUT_DIM], wh4[:, k, :], g4[:, k:k + 1]
                )
                nc.vector.tensor_copy(
                    out=G[:, t, OUT_DIM:GW], in_=g4[:, k:k + 1]
                )

        # ---- phase 2 (transposed): nmT[o, i] += G[jc, o].T @ AT[jc, i] ----
        # numerator rows o=0..127, denominator from the g column (m=1 matmul)
        nm = [ps.tile([P, HF], F32, tag=f"nm{h}", name=f"nm{h}", bufs=1)
              for h in range(2)]
        dn = [ps.tile([P, HF], F32, tag=f"dn{h}", name=f"dn{h}", bufs=1)
              for h in range(2)]
        for c in range(NJ):
            at_sb = atp.tile([P, RPC], PH2_DT, tag="at_sb", name="at_sb")
            nc.sync.dma_start(out=at_sb, in_=at[c * P:(c + 1) * P, :])
            for h in range(2):
                nc.tensor.matmul(
                    nm[h][:, :],
                    G[:, c, 0:OUT_DIM],
                    at_sb[:, h * HF:(h + 1) * HF],
                    start=(c == 0),
                    stop=(c == NJ - 1),
                )
            for h in range(2):
                nc.tensor.matmul(
                    dn[h][0:1, :],
                    G[:, c, OUT_DIM:GW],
                    at_sb[:, h * HF:(h + 1) * HF],
                    start=(c == 0),
                    stop=(c == NJ - 1),
                )
        for h in range(2):
            rc_sb = outp.tile([1, HF], F32, tag="rc", name="rc")
            nc.vector.reciprocal(rc_sb, dn[h][0:1, :])
            rel = outp.tile([P, HF], F32, tag="rel", name="rel")
            nc.scalar.activation(rel, nm[h], AFT.Relu)
            rbc = ps.tile([P, HF], F32, tag="rbc", name="rbc", bufs=1)
            nc.tensor.matmul(rbc, ones[0:1, 0:P], rc_sb, start=True, stop=True)
            o_sb = outp.tile([P, HF], F32, tag="osb", name="osb")
            nc.vector.tensor_mul(o_sb, rel, rbc)
            nc.sync.dma_start(out=out[:, h * HF:(h + 1) * HF], in_=o_sb)


def build_nc(repeat=1):
    nc = bacc.Bacc("TRN2", target_bir_lowering=False)
    at = nc.dram_tensor("at", [N, RPC], PH2_DT, kind="ExternalInput")     # A.T col-block
    xt = nc.dram_tensor("xt", [IN_DIM, N], F32, kind="ExternalInput")     # X.T (full)
    wt = nc.dram_tensor("wt", [IN_DIM, OUT_DIM], F32, kind="ExternalInput")  # W.T
    awr = nc.dram_tensor("awr", [P, B1 * OUT_DIM], F32, kind="ExternalInput")  # a_w tiled
    out = nc.dram_tensor("out", [OUT_DIM, RPC], F32, kind="ExternalOutput")  # transposed

    with tile.TileContext(nc) as tc:
        with (
            tc.tile_pool(name="big", bufs=1) as big,
            tc.tile_pool(name="atp", bufs=16) as atp,
            tc.tile_pool(name="ph1", bufs=4) as ph1,
            tc.tile_pool(name="outp", bufs=2) as outp,
        ):
            for _ in range(repeat):
                emit_body(nc, tc, (at, xt, wt, awr, out), (big, atp, ph1, outp))
    nc.compile()
    return nc


_NC_CACHE = None


def _get_nc():
    global _NC_CACHE
    if _NC_CACHE is None:
        _NC_CACHE = build_nc()
    return _NC_CACHE


def make_in_maps(X, A, W, a_w):
    X = np.ascontiguousarray(np.asarray(X, dtype=np.float32))
    A = np.ascontiguousarray(np.asarray(A, dtype=np.float32))
    W = np.ascontiguousarray(np.asarray(W, dtype=np.float32))
    a_w = np.ascontiguousarray(np.asarray(a_w, dtype=np.float32))

    xt = np.ascontiguousarray(X.T)                       # [256, 8192]
    wt = np.ascontiguousarray(W.T)                       # [256, 128]
    awr = np.ascontiguousarray(np.broadcast_to(np.tile(a_w, B1)[None, :], (P, B1 * OUT_DIM)))

    at_np = mybir.dt.np(PH2_DT)
    in_maps = []
    for c in range(NCORES):
        atb = np.ascontiguousarray(A[c * RPC:(c + 1) * RPC, :].T.astype(at_np))
        in_maps.append({"at": atb, "xt": xt, "wt": wt, "awr": awr})
    return in_maps


def kernel_with_results(X, A, W, a_w, trace=False):
    in_maps = make_in_maps(X, A, W, a_w)
    res = run_bass_kernel_spmd(_get_nc(), in_maps, list(range(NCORES)), trace=trace)
    out = np.concatenate(
        [np.ascontiguousarray(r["out"].T) for r in res.results], axis=0
    )
    return out.astype(np.float32), res


def kernel(X, A, W, a_w):
    out, _ = kernel_with_results(X, A, W, a_w)
    return out


# revision 41
# speedup vs baseline: 12.9625x; 1.4386x over previous
"""GATv2 layer (broadcast-score variant) as a Bass/Tile kernel on 8 NeuronCores.

Math: since scores[i,j] = e[j] (row-broadcast) masked by A, the masked softmax +
aggregation collapse to
    g = exp(e),  e = relu(X @ W.T) @ a_w
    out = relu( (A @ (g*Wh)) / (A @ g) )          with Wh = X @ W.T
Each core computes a 1024-row block of the output:
  phase 1 (replicated): Wh, e, g, G = [g*Wh | g]  ([8192, 129])
  phase 2 (sharded):    acc = A_block @ G  via PE, contraction j on partitions,
                        using the host-transposed A.T block as lhsT.
"""

import numpy as np

import concourse.tile as tile
from concourse import bacc, mybir
from concourse.bass_utils import run_bass_kernel_spmd

N, IN_DIM, OUT_DIM = 8192, 256, 128
NCORES = 8
RPC = N // NCORES          # rows per core (1024)
P = 128                    # partitions
NJ = N // P                # 64 contraction chunks
NI = RPC // P              # 8 output row-tiles per core
DH = IN_DIM // P           # 2 chunks of the d-contraction
import os

F32 = mybir.dt.float32
F32R = mybir.dt.float32r   # TF32-like: 1 cyc/row on PE when moving dim >= 256
BF16 = mybir.dt.bfloat16
# phase-2 matmul dtype: f32r, f32, bf16
PH2_DT = {"f32r": F32R, "f32": F32, "bf16": BF16}[os.environ.get("PH2_DT", "bf16")]
# phase-1 (Wh) matmul dtype; f32r pads the WT rhs to 256 cols for full rate
PH1_DT = {"f32": F32, "bf16": BF16, "f32r": F32R}[os.environ.get("PH1_DT", "bf16")]
WTW = 256 if PH1_DT == F32R else OUT_DIM   # wt width (f32r needs N>=256)
AFT = mybir.ActivationFunctionType
GW = OUT_DIM + 1           # 129 G columns (gWh | g)
GP = 132                   # G pitch (16B aligned)
B1 = 2                     # phase-1 j-tile batch
NB = NJ // B1
HF = RPC // 2              # 512-wide i-halves for phase-2 streams


def emit_body(nc, tc, io, pools):
    at, xt, wt, awr, out = io
    big, atp, ph1, outp = pools

    xt_sb = big.tile([P, DH, N], PH1_DT, tag="xt_sb", name="xt_sb")
    nc.sync.dma_start(out=xt_sb, in_=xt.rearrange("(dh p) n -> p dh n", p=P))
    wt_sb = big.tile([P, DH, WTW], PH1_DT, tag="wt_sb", name="wt_sb")
    nc.sync.dma_start(out=wt_sb, in_=wt.rearrange("(dh p) o -> p dh o", p=P))
    aw_sb = big.tile([P, B1, OUT_DIM], F32, tag="aw_sb", name="aw_sb")
    nc.sync.dma_start(out=aw_sb, in_=awr.rearrange("p (b o) -> p b o", b=B1))
    G = big.tile([P, NJ, GP], PH2_DT, tag="G", name="G")
    ones = big.tile([1, P], F32, tag="ones", name="ones")
    nc.vector.memset(ones, 1.0)

    with tc.tile_pool(name="ps", bufs=1, space="PSUM") as ps:
        # ---- phase 1: Wh tiles -> e -> g -> G = [g*Wh | g], batched by B1 ----
        for b in range(NB):
            wh4 = ps.tile([P, B1, WTW], F32, tag="wh4", name="wh4", bufs=2)
            for k in range(B1):
                t = b * B1 + k
                for dh in range(DH):
                    nc.tensor.matmul(
                        wh4[:, k, :],
                        xt_sb[:, dh, t * P:(t + 1) * P],
                        wt_sb[:, dh, :],
                        start=(dh == 0),
                        stop=(dh == DH - 1),
                    )
            r4 = ph1.tile([P, B1, OUT_DIM], F32, name="r4")
            nc.scalar.activation(r4, wh4[:, :, 0:OUT_DIM], AFT.Relu)
            m4 = ph1.tile([P, B1, OUT_DIM], F32, name="m4")
            nc.vector.tensor_mul(m4, r4, aw_sb)
            e4 = ph1.tile([P, B1], F32, name="e4")
            nc.vector.reduce_sum(e4, m4, axis=mybir.AxisListType.X)
            g4 = ph1.tile([P, B1], F32, name="g4")
            nc.scalar.activation(g4, e4, AFT.Exp)
            for k in range(B1):
                t = b * B1 + k
                nc.vector.tensor_scalar_mul(
                    G[:, t, 0:OUT_DIM], wh4[:, k, 0:OUT_DIM], g4[:, k:k + 1]
                )
                nc.vector.tensor_copy(
                    out=G[:, t, OUT_DIM:GW], in_=g4[:, k:k + 1]
                )

        # ---- phase 2 (transposed): nmT[o, i] += G[jc, o].T @ AT[jc, i] ----
        # numerator rows o=0..127, denominator from the g column (m=1 matmul)
        nm = [ps.tile([P, HF], F32, tag=f"nm{h}", name=f"nm{h}", bufs=1)
              for h in range(2)]
        dn = [ps.tile([P, HF], F32, tag=f"dn{h}", name=f"dn{h}", bufs=1)
              for h in range(2)]
        for c in range(NJ):
            at_sb = atp.tile([P, RPC], PH2_DT, tag="at_sb", name="at_sb")
            nc.sync.dma_start(out=at_sb, in_=at[c * P:(c + 1) * P, :])
            for h in range(2):
                nc.tensor.matmul(
                    nm[h][:, :],
                    G[:, c, 0:OUT_DIM],
                    at_sb[:, h * HF:(h + 1) * HF],
                    start=(c == 0),
                    stop=(c == NJ - 1),
                )
            for h in range(2):
                nc.tensor.matmul(
                    dn[h][0:1, :],
                    G[:, c, OUT_DIM:GW],
                    at_sb[:, h * HF:(h + 1) * HF],
                    start=(c == 0),
                    stop=(c == NJ - 1),
                )
        for h in range(2):
            rc_sb = outp.tile([1, HF], F32, tag="rc", name="rc")
            nc.vector.reciprocal(rc_sb, dn[h][0:1, :])
            rel = outp.tile([P, HF], F32, tag="rel", name="rel")
            nc.scalar.activation(rel, nm[h], AFT.Relu)
            rbc = ps.tile([P, HF], F32, tag="rbc", name="rbc", bufs=1)
            nc.tensor.matmul(rbc, ones[0:1, 0:P], rc_sb, start=True, stop=True)
            o_sb = outp.tile([P, HF], F32, tag="osb", name="osb")
            nc.vector.tensor_mul(o_sb, rel, rbc)
            nc.sync.dma_start(out=out[:, h * HF:(h + 1) * HF], in_=o_sb)


def build_nc(repeat=1):
    nc = bacc.Bacc("TRN2", target_bir_lowering=False)
    at = nc.dram_tensor("at", [N, RPC], PH2_DT, kind="ExternalInput")     # A.T col-block
    xt = nc.dram_tensor("xt", [IN_DIM, N], PH1_DT, kind="ExternalInput")  # X.T (full)
    wt = nc.dram_tensor("wt", [IN_DIM, WTW], PH1_DT, kind="ExternalInput")  # W.T (maybe padded)
    awr = nc.dram_tensor("awr", [P, B1 * OUT_DIM], F32, kind="ExternalInput")  # a_w tiled
    out = nc.dram_tensor("out", [OUT_DIM, RPC], F32, kind="ExternalOutput")  # transposed

    with tile.TileContext(nc) as tc:
        with (
            tc.tile_pool(name="big", bufs=1) as big,
            tc.tile_pool(name="atp", bufs=16) as atp,
            tc.tile_pool(name="ph1", bufs=4) as ph1,
            tc.tile_pool(name="outp", bufs=2) as outp,
        ):
            for _ in range(repeat):
                emit_body(nc, tc, (at, xt, wt, awr, out), (big, atp, ph1, outp))
    nc.compile()
    return nc


_NC_CACHE = None


def _get_nc():
    global _NC_CACHE
    if _NC_CACHE is None:
        _NC_CACHE = build_nc()
    return _NC_CACHE


def make_in_maps(X, A, W, a_w):
    X = np.ascontiguousarray(np.asarray(X, dtype=np.float32))
    A = np.ascontiguousarray(np.asarray(A, dtype=np.float32))
    W = np.ascontiguousarray(np.asarray(W, dtype=np.float32))
    a_w = np.ascontiguousarray(np.asarray(a_w, dtype=np.float32))

    ph1_np = mybir.dt.np(PH1_DT)
    xt = np.ascontiguousarray(X.T.astype(ph1_np))        # [256, 8192]
    wt_full = np.zeros((IN_DIM, WTW), dtype=np.float32)
    wt_full[:, :OUT_DIM] = W.T
    wt = np.ascontiguousarray(wt_full.astype(ph1_np))    # [256, WTW]
    awr = np.ascontiguousarray(np.broadcast_to(np.tile(a_w, B1)[None, :], (P, B1 * OUT_DIM)))

    at_np = mybir.dt.np(PH2_DT)
    in_maps = []
    for c in range(NCORES):
        atb = np.ascontiguousarray(A[c * RPC:(c + 1) * RPC, :].T.astype(at_np))
        in_maps.append({"at": atb, "xt": xt, "wt": wt, "awr": awr})
    return in_maps


def kernel_with_results(X, A, W, a_w, trace=False):
    in_maps = make_in_maps(X, A, W, a_w)
    res = run_bass_kernel_spmd(_get_nc(), in_maps, list(range(NCORES)), trace=trace)
    out = np.concatenate(
        [np.ascontiguousarray(r["out"].T) for r in res.results], axis=0
    )
    return out.astype(np.float32), res


def kernel(X, A, W, a_w):
    out, _ = kernel_with_results(X, A, W, a_w)
    return out
